# revision 2
# baseline (speedup 1.0000x reference)
"""CKAN GNN message-passing kernel for Trainium2 (8 NeuronCores, SPMD).

Layout notes (per core, B_loc=256):
  - 8 "quarters" q = side*4 + hop*2 + btile, each 128 b-rows x 64 m = 8192 edges.
  - Entity gathers land b-major via ONE indirect DMA per tile (8192 row
    descriptors in a single SWDGE call): tile[p=b, m*64+d].
  - The relation term r1T is streamed pre-transposed from the host:
    r1T[e*64+f, pj*128+b] = G1[rel[b, 2pj+e]][f]  (pair-stacked, column-major),
    so only the head tiles need PE transposes.
  - MLP runs column-major (feature on partitions) on [128f, 128b] pair blocks;
    w1/w2 are block-diagonal duplicated so one matmul covers both pair slices.
"""

import functools
import sys

import numpy as np

sys.path.insert(0, "/opt/trn_rl_repo")

N_ENT, N_REL, DIM, L, B, M = 500000, 32, 64, 2, 2048, 64
NCORES = 8
BL = B // NCORES        # 256
NBT = BL // 128         # 2 b-tiles
NQ = 8                  # quarters per core
F16 = np.float16


@functools.lru_cache(maxsize=4)
def _build(rep=1, mode="full"):
    TIER = {"gather": 0, "trans": 1, "l1": 2, "l2": 3, "full": 4}[mode]
    import concourse.bacc as bacc
    import concourse.bass as bass
    import concourse.mybir as mybir
    import concourse.tile as tile

    f16 = mybir.dt.float16
    f32 = mybir.dt.float32
    i32 = mybir.dt.int32
    AF = mybir.ActivationFunctionType
    OP = mybir.AluOpType

    nc = bacc.Bacc("TRN2", target_bir_lowering=False, debug=False)

    ent16 = nc.dram_tensor("ent16", [N_ENT, DIM], f16, kind="ExternalInput").ap()
    rec16 = nc.dram_tensor("rec16", [N_ENT, DIM], f16, kind="ExternalInput").ap()
    r1 = nc.dram_tensor("r1", [NQ, 128, 64 * DIM], f16, kind="ExternalInput").ap()
    idx_h = nc.dram_tensor("idx_h", [NQ, 128, 64], i32, kind="ExternalInput").ap()
    idx_t = nc.dram_tensor("idx_t", [NQ, 128, 64], i32, kind="ExternalInput").ap()
    idx_e0 = nc.dram_tensor("idx_e0", [4, 128, 64], i32, kind="ExternalInput").ap()
    w1hblk = nc.dram_tensor("w1hblk", [128, 128], f16, kind="ExternalInput").ap()
    iiblk = nc.dram_tensor("iiblk", [128, 128], f16, kind="ExternalInput").ap()
    w2blk = nc.dram_tensor("w2blk", [128, 128], f16, kind="ExternalInput").ap()
    w3s = nc.dram_tensor("w3s", [128, 2], f16, kind="ExternalInput").ap()
    waggt = nc.dram_tensor("waggt", [192, 64], f16, kind="ExternalInput").ap()
    w3a = nc.dram_tensor("w3a", [128, 1], f32, kind="ExternalInput").ap()
    b2w3 = nc.dram_tensor("b2w3", [128, 1], f32, kind="ExternalInput").ap()
    b3w = nc.dram_tensor("b3w", [128, 1], f32, kind="ExternalInput").ap()
    baggh = nc.dram_tensor("baggh", [64, 1], f32, kind="ExternalInput").ap()
    idf = nc.dram_tensor("idf", [128, 128], f16, kind="ExternalInput").ap()
    ones16 = nc.dram_tensor("ones16", [64, 1], f16, kind="ExternalInput").ap()
    zeroc = nc.dram_tensor("zeroc", [128, 1], f32, kind="ExternalInput").ap()
    halfc = nc.dram_tensor("halfc", [128, 1], f32, kind="ExternalInput").ap()
    outv = nc.dram_tensor("outv", [BL, 1], f32, kind="ExternalOutput").ap()
    dbge = nc.dram_tensor("dbge", [4, 128, 192], f16, kind="ExternalOutput").ap()

    with tile.TileContext(nc) as tc:
        with (
            tc.tile_pool(name="consts", bufs=1) as pc,
            tc.tile_pool(name="idx", bufs=2) as pidx,
            tc.tile_pool(name="xg", bufs=2) as pxg,
            tc.tile_pool(name="tails", bufs=2) as ptg,
            tc.tile_pool(name="e0", bufs=2) as pe0,
            tc.tile_pool(name="xsb", bufs=2) as pxs,
            tc.tile_pool(name="ysb", bufs=2) as pys,
            tc.tile_pool(name="wsb", bufs=2) as pws,
            tc.tile_pool(name="small", bufs=2) as psm,
            tc.tile_pool(name="ecat", bufs=4) as pec,
            tc.tile_pool(name="pX", bufs=2, space="PSUM") as ppx,
            tc.tile_pool(name="pY", bufs=2, space="PSUM") as ppy,
            tc.tile_pool(name="pPi", bufs=1, space="PSUM") as pppi,
        ):
            # ---- load constants to SBUF
            def cload(ap_dram, shape, dt):
                t = pc.tile(shape, dt, tag=ap_dram.tensor.name)
                nc.sync.dma_start(t[:], ap_dram)
                return t

            w1hb = cload(w1hblk, [128, 128], f16)
            iib = cload(iiblk, [128, 128], f16)
            w2b = cload(w2blk, [128, 128], f16)
            w3ss = cload(w3s, [128, 2], f16)
            wag1 = pc.tile([96, 64], f16, tag="wag1")
            nc.sync.dma_start(wag1[:], waggt[0:96, :])
            wag2 = pc.tile([96, 64], f16, tag="wag2")
            nc.sync.dma_start(wag2[:], waggt[96:192, :])
            w3as = cload(w3a, [128, 1], f32)
            b2w3s = cload(b2w3, [128, 1], f32)
            b3ws = cload(b3w, [128, 1], f32)
            bgs = cload(baggh, [64, 1], f32)
            idt = cload(idf, [128, 128], f16)
            on1 = cload(ones16, [64, 1], f16)
            zc = cload(zeroc, [128, 1], f32)
            hc = cload(halfc, [128, 1], f32)

            ecats = {}
            uos = {}

            for _rep in range(rep):
              for q in range(NQ):
                s, rem = divmod(q, 4)
                l, t = divmod(rem, 2)

                ih = pidx.tile([128, 64], i32, tag="ih")
                nc.sync.dma_start(ih[:], idx_h[q])
                it = pidx.tile([128, 64], i32, tag="it")
                nc.sync.dma_start(it[:], idx_t[q])

                # gathered heads (fp16 rows), one indirect DMA for all 8192 rows
                hg = pxg.tile([128, 64 * DIM], f16, tag="hg")
                nc.gpsimd.indirect_dma_start(
                    out=hg[:, :],
                    out_offset=None,
                    in_=ent16,
                    in_offset=bass.IndirectOffsetOnAxis(ap=ih[:, :], axis=0),
                )
                # host relation term, pre-transposed pair-stacked layout
                r1g = pxg.tile([128, 64 * DIM], f16, tag="r1g")
                nc.sync.dma_start(r1g[:, :], r1[q])

                tg = ptg.tile([128, 64 * DIM], f16, tag="tg")
                nc.gpsimd.indirect_dma_start(
                    out=tg[:, :],
                    out_offset=None,
                    in_=ent16,
                    in_offset=bass.IndirectOffsetOnAxis(ap=it[:, :], axis=0),
                )

                if l == 0:
                    # e0 gather + tree-mean (scale folded into waggt rows 0:64)
                    ie = pidx.tile([128, 64], i32, tag="ie")
                    nc.sync.dma_start(ie[:], idx_e0[s * NBT + t])
                    e0g = pe0.tile([128, 64 * DIM], f16, tag="e0g")
                    nc.gpsimd.indirect_dma_start(
                        out=e0g[:, :],
                        out_offset=None,
                        in_=(rec16 if s == 0 else ent16),
                        in_offset=bass.IndirectOffsetOnAxis(ap=ie[:, :], axis=0),
                    )
                    ecat = pec.tile([128, 3 * DIM], f16, tag=f"ecat{s}{t}")
                    ecats[(s, t)] = ecat
                    if TIER >= 1:
                        w = 64 * DIM
                        while w > 2 * DIM:
                            nc.vector.tensor_tensor(
                                out=e0g[:, 0 : w // 2],
                                in0=e0g[:, 0 : w // 2],
                                in1=e0g[:, w // 2 : w],
                                op=OP.add,
                            )
                            w //= 2
                        nc.vector.tensor_tensor(
                            out=ecat[:, 0:DIM],
                            in0=e0g[:, 0:DIM],
                            in1=e0g[:, DIM : 2 * DIM],
                            op=OP.add,
                        )
                else:
                    ecat = ecats[(s, t)]

                if TIER < 1:
                    continue

                # ---- transposes: head pair pj -> X[:, pj*128 : +128]
                xsb = pxs.tile([128, 32 * 128], f16, tag="xsb")
                for cc in range(8):
                    pX = ppx.tile([128, 512], f16, space="PSUM", tag="pX")
                    for u in range(4):
                        pj = 4 * cc + u
                        nc.tensor.matmul(
                            pX[:, u * 128 : (u + 1) * 128],
                            lhsT=hg[:, pj * 128 : (pj + 1) * 128],
                            rhs=idt[:],
                            is_transpose=True,
                        )
                    # X copy: split between ACT and DVE
                    dst = xsb[:, cc * 512 : (cc + 1) * 512]
                    if cc % 2 == 0:
                        nc.scalar.copy(dst, pX[:])
                    else:
                        nc.vector.tensor_copy(dst, pX[:])

                if TIER < 2:
                    continue
                # ---- L1/L2/L3 in chunks of 8 pair-blocks (1024 X cols)
                y1sb = pys.tile([128, 4096], f16, tag="y1sb")
                y2sb = None
                pPi = None
                if TIER >= 3:
                    y2sb = pys.tile([128, 4096], f16, tag="y2sb")
                if TIER >= 4:
                    pPi = pppi.tile([128, 64], f32, space="PSUM", tag="pPi")
                for c in range(4):
                    pY1 = ppy.tile([128, 1024], f32, space="PSUM", tag="pY")
                    for g2 in range(2):
                        sl = slice(c * 1024 + g2 * 512, c * 1024 + (g2 + 1) * 512)
                        nc.tensor.matmul(
                            pY1[:, g2 * 512 : (g2 + 1) * 512],
                            lhsT=w1hb[:],
                            rhs=xsb[:, sl],
                            start=True, stop=False,
                        )
                        nc.tensor.matmul(
                            pY1[:, g2 * 512 : (g2 + 1) * 512],
                            lhsT=iib[:],
                            rhs=r1g[:, sl],
                            start=False, stop=True,
                        )
                    nc.vector.tensor_scalar(
                        out=y1sb[:, c * 1024 : (c + 1) * 1024],
                        in0=pY1[:],
                        scalar1=0.0,
                        scalar2=None,
                        op0=OP.max,
                    )
                    if TIER < 3:
                        continue
                    pY2 = ppy.tile([128, 1024], f32, space="PSUM", tag="pY")
                    for g2 in range(2):
                        nc.tensor.matmul(
                            pY2[:, g2 * 512 : (g2 + 1) * 512],
                            lhsT=w2b[:],
                            rhs=y1sb[:, c * 1024 + g2 * 512 : c * 1024 + (g2 + 1) * 512],
                        )
                    # y2w = relu(|w3| z + |w3| b2) = |w3| relu(z + b2)
                    nc.scalar.activation(
                        y2sb[:, c * 1024 : (c + 1) * 1024], pY2[:], AF.Relu,
                        bias=b2w3s[:, 0:1], scale=w3as[:, 0:1],
                    )
                    if TIER < 4:
                        continue
                    for u in range(8):
                        k = 8 * c + u
                        nc.tensor.matmul(
                            pPi[:, 2 * k : 2 * k + 2],
                            lhsT=y2sb[:, k * 128 : (k + 1) * 128],
                            rhs=w3ss[:],
                        )

                if TIER < 4:
                    continue
                # ---- softmax (order-free over m-cols), exp_and_others set
                tq = psm.tile([128, 64], f16, tag="tq")
                nc.scalar.activation(tq[:], pPi[:], AF.Tanh, bias=b3ws[:, 0:1], scale=0.5)
                esb = psm.tile([128, 64], f16, tag="esb")
                nc.scalar.activation(esb[:], tq[:], AF.Exp, bias=hc[:, 0:1], scale=0.5)
                zs = psm.tile([128, 1], f32, tag="zs")
                nc.vector.reduce_sum(zs[:], esb[:], axis=mybir.AxisListType.X)
                rz = psm.tile([128, 1], f32, tag="rz")
                nc.vector.reciprocal(rz[:], zs[:])
                esn = psm.tile([128, 64], f16, tag="esn")
                nc.vector.tensor_scalar_mul(esn[:], esb[:], rz[:, 0:1])

                # ---- weighted sum of tails: W = tg * esn[..., bcast d] ; tree over m
                wmul = pws.tile([128, 64 * DIM], f16, tag="wmul")
                tg3 = tg[:].rearrange("p (m d) -> p m d", d=DIM)
                esn3 = esn[:].rearrange("p (m o) -> p m o", o=1).to_broadcast([128, 64, DIM])
                nc.vector.tensor_tensor(
                    out=wmul[:].rearrange("p (m d) -> p m d", d=DIM),
                    in0=tg3,
                    in1=esn3,
                    op=OP.mult,
                )
                w = 64 * DIM
                while w > 2 * DIM:
                    nc.vector.tensor_tensor(
                        out=wmul[:, 0 : w // 2],
                        in0=wmul[:, 0 : w // 2],
                        in1=wmul[:, w // 2 : w],
                        op=OP.add,
                    )
                    w //= 2
                nc.vector.tensor_tensor(
                    out=ecat[:, DIM + l * DIM : 2 * DIM + l * DIM],
                    in0=wmul[:, 0:DIM],
                    in1=wmul[:, DIM : 2 * DIM],
                    op=OP.add,
                )

                # ---- per (side, btile) aggregation after hop 1
                if l == 1:
                    nc.sync.dma_start(dbge[s * NBT + t], ecat[:])
                    pA = ppy.tile([64, 128], f32, space="PSUM", tag="pY")
                    for half in range(2):
                        peT = ppx.tile([96, 128], f16, space="PSUM", tag="pX")
                        nc.tensor.matmul(
                            peT[:],
                            lhsT=ecat[:, half * 96 : (half + 1) * 96],
                            rhs=idt[:],
                            is_transpose=True,
                        )
                        eT = psm.tile([96, 128], f16, tag="eT")
                        nc.vector.tensor_copy(eT[:], peT[:])
                        nc.tensor.matmul(
                            pA[:],
                            lhsT=(wag1 if half == 0 else wag2)[:],
                            rhs=eT[:],
                            start=(half == 0),
                            stop=(half == 1),
                        )
                    ta = psm.tile([64, 128], f16, tag="ta")
                    nc.scalar.activation(ta[:], pA[:], AF.Tanh, bias=bgs[:, 0:1], scale=0.5)
                    uo = psm.tile([64, 128], f16, tag=f"uo{s}{t}")
                    nc.scalar.activation(uo[:], ta[:], AF.Copy, bias=0.5, scale=0.5)
                    uos[(s, t)] = uo
                    if s == 1:
                        prod = psm.tile([64, 128], f16, tag="prod")
                        nc.vector.tensor_tensor(
                            out=prod[:], in0=uos[(0, t)][:], in1=uo[:], op=OP.mult
                        )
                        pD = ppy.tile([128, 1], f32, space="PSUM", tag="pY")
                        nc.tensor.matmul(pD[:], lhsT=prod[:], rhs=on1[:])
                        td = psm.tile([128, 1], f32, tag="td")
                        nc.scalar.activation(td[:], pD[:], AF.Tanh, bias=zc[:, 0:1], scale=0.5)
                        ob = psm.tile([128, 1], f32, tag="ob")
                        nc.scalar.activation(ob[:], td[:], AF.Copy, bias=0.5, scale=0.5)
                        nc.sync.dma_start(outv[t * 128 : (t + 1) * 128, :], ob[:])

            if TIER < 4:
                for t in range(NBT):
                    nc.sync.dma_start(outv[t * 128 : (t + 1) * 128, :], zc[:])

    nc.compile()
    return nc


def _prep_inputs(inputs):
    """Host-side packing. Returns (shared, per_core list of dicts)."""
    gi = lambda k: np.asarray(inputs[k])
    ent_emb = np.asarray(inputs["ent_emb"], np.float32)
    rec_emb = np.asarray(inputs["rec_emb"], np.float32)
    rel_emb = np.asarray(inputs["rel_emb"], np.float32)
    w1 = np.asarray(inputs["w1"], np.float32)
    b1 = np.asarray(inputs["b1"], np.float32)
    w2 = np.asarray(inputs["w2"], np.float32)
    b2 = np.asarray(inputs["b2"], np.float32)
    w3 = np.asarray(inputs["w3"], np.float32)
    b3 = np.asarray(inputs["b3"], np.float32)
    wagg = np.asarray(inputs["wagg"], np.float32)
    bagg = np.asarray(inputs["bagg"], np.float32)

    ent16 = np.ascontiguousarray(ent_emb, F16) if ent_emb.dtype == F16 else ent_emb.astype(F16)
    rec16 = rec_emb.astype(F16)
    G1 = (rel_emb @ w1[:, DIM:].T + b1).astype(F16)          # [32, 64]

    w1hblk = np.zeros((128, 128), np.float32)
    w1hblk[0:64, 0:64] = w1[:, :DIM].T
    w1hblk[64:128, 64:128] = w1[:, :DIM].T
    w1hblk = w1hblk.astype(F16)
    iiblk = np.zeros((128, 128), np.float32)
    iiblk[0:64, 0:64] = np.eye(64)
    iiblk[64:128, 64:128] = np.eye(64)
    iiblk = iiblk.astype(F16)
    w2blk = np.zeros((128, 128), np.float32)
    w2blk[0:64, 0:64] = w2.T
    w2blk[64:128, 64:128] = w2.T
    w2blk = w2blk.astype(F16)
    w3s = np.zeros((128, 2), np.float32)
    w3s[0:64, 0] = np.sign(w3[0])
    w3s[64:128, 1] = np.sign(w3[0])
    w3s = w3s.astype(F16)
    w3a = np.concatenate([np.abs(w3[0]), np.abs(w3[0])])[:, None].astype(np.float32)
    b2w3 = (np.concatenate([b2, b2]) * np.concatenate([np.abs(w3[0]), np.abs(w3[0])]))[:, None].astype(np.float32)
    waggt = wagg.T.copy()
    waggt[0:DIM, :] /= 64.0                                   # fold e0 mean
    waggt = waggt.astype(F16)
    b3w = np.full((128, 1), b3[0] / 2.0, np.float32)
    baggh = (bagg / 2.0)[:, None].astype(np.float32)
    idf = np.eye(128, dtype=F16)
    ones16 = np.ones((64, 1), F16)
    zeroc = np.zeros((128, 1), np.float32)
    halfc = np.full((128, 1), 0.5, np.float32)

    shared = dict(ent16=ent16, rec16=rec16, w1hblk=w1hblk, iiblk=iiblk, w2blk=w2blk, w3s=w3s,
                  w3a=w3a, b2w3=b2w3, waggt=waggt, b3w=b3w, baggh=baggh, idf=idf,
                  ones16=ones16, zeroc=zeroc, halfc=halfc)

    u_ent, v_ent = gi("u_ent"), gi("v_ent")
    hs = [gi("u_heads"), gi("v_heads")]
    rs = [gi("u_rels"), gi("v_rels")]
    ts = [gi("u_tails"), gi("v_tails")]

    G1f = G1.astype(np.float32)
    per_core = []
    for k in range(NCORES):
        r1 = np.empty((NQ, 128, 64 * DIM), F16)
        idx_h = np.empty((NQ, 128, 64), np.int32)
        idx_t = np.empty((NQ, 128, 64), np.int32)
        idx_e0 = np.empty((4, 128, 64), np.int32)
        for q in range(NQ):
            s, rem = divmod(q, 4)
            l, t = divmod(rem, 2)
            bsl = slice(k * BL + t * 128, k * BL + (t + 1) * 128)
            idx_h[q] = hs[s][l, bsl, :]
            idx_t[q] = ts[s][l, bsl, :]
            # pre-transposed pair-stacked relation term:
            # r1[q][e*64+f, pj*128+b] = G1[rel[b, 2pj+e]][f]
            A = G1f[rs[s][l, bsl, :]]                        # [128b, 64m, 64f]
            r1[q] = (A.reshape(128, 32, 2, 64)
                      .transpose(2, 3, 1, 0)
                      .reshape(128, 64 * DIM)).astype(F16)
        for s in range(2):
            for t in range(NBT):
                bsl = slice(k * BL + t * 128, k * BL + (t + 1) * 128)
                idx_e0[s * NBT + t] = (u_ent if s == 0 else v_ent)[bsl, :]
        per_core.append(dict(r1=r1, idx_h=idx_h, idx_t=idx_t, idx_e0=idx_e0))
    return shared, per_core


def kernel(**inputs) -> np.ndarray:
    from concourse.bass_utils import run_bass_kernel_spmd

    nc = _build()
    shared, per_core = _prep_inputs(inputs)
    in_maps = [{**shared, **pc} for pc in per_core]
    res = run_bass_kernel_spmd(nc, in_maps, core_ids=list(range(NCORES)))
    out = np.concatenate([res.results[k]["outv"][:, 0] for k in range(NCORES)])
    return out.astype(np.float32)
